# revision 1
# baseline (speedup 1.0000x reference)
"""DepthConv kernel for Trainium2 (Bass/Tile), data-parallel over batch on 8 cores.

Problem: out[b,o,x,y] = sum_{c,k} w[o,c,k] * data[b,c,x+i,y+j] * aff[b,k,x,y]
         aff[b,k,x,y] = exp(-8.3*|depth[b,x+i,y+j] - depth[b,x+1,y+1]|), k=(i,j) in 3x3
Shapes: data [8,16,256,256], depth [8,1,256,256], weight [16,16,3,3] -> out [8,16,254,254]

Per-core layout (1 image/core): partitions = (strip q=0..7, channel c=0..15).
Each strip covers 32 output rows; free dim n = xl*256+y (flat, row-wrapping).
 - 3x3 taps become pure free-dim shifts (i*256+j) of one resident data tile.
 - Per-tap matmul uses block-diagonal weights [(q,c),(q,o)] so all 8 strips'
   channel contractions run in one full-width 128x128 matmul; 9 taps
   PSUM-accumulate.
 - Affinity aff[(q,k),n] is computed per n-tile (PE center-selection matmul +
   DVE sub + ACT abs/exp), then replicated across the 16 channel rows of each
   strip via a selection-matrix matmul on the PE (output straight into PSUM,
   consumed by the DVE multiply).
 - float32r matmuls (full PE rate at N=512, fp32 storage).
 - The entire input (data windows, pre-shifted depth taps, weight/selection
   matrices) is packed host-side into ONE [128, TOT] tensor loaded by ONE DMA,
   and each tile stores with ONE DMA — keeps every instruction's semaphore
   wait count within walrus's tiny per-instruction limits.
"""

import numpy as np

B, C, H, W = 8, 16, 256, 256
O, KH, KW = 16, 3, 3
ALPHA = 8.3
OH, OW = H - KH + 1, W - KW + 1  # 254, 254
P = 128
NQ, QROWS = 8, 32           # strips, output rows per strip
NFREE = QROWS * W           # 8192 flat pixels per strip (incl. y>=254 garbage)
NTILE = 512
NT = NFREE // NTILE         # 16 n-tiles (2 output rows each)
DWIN = 34 * W + 16          # data window: 34 rows halo + shift pad
TAPS = [(i, j) for i in range(KH) for j in range(KW)]
NC_KS = [k for k in range(9) if k != 4]  # non-center taps
NBLK = 18                   # 9 weight blocks + 8 tap-select + 1 center-select
D0 = 0                      # data window offset in the packed tensor
Z0 = DWIN                   # dep_t offset
M0 = DWIN + NFREE           # wsmat offset
TOT = DWIN + NFREE + NBLK * P

_CACHE = {}


def _build_nc():
    import concourse.bass as bass
    import concourse.bacc as bacc
    import concourse.mybir as mybir
    from concourse.tile import TileContext
    from concourse.alu_op_type import AluOpType
    from concourse.bass_types import AP

    f32 = mybir.dt.float32
    f32r = mybir.dt.float32r
    f16 = mybir.dt.float16
    AF = mybir.ActivationFunctionType

    nc = bacc.Bacc(None, target_bir_lowering=False)
    allin_d = nc.dram_tensor("allin", [P, TOT], f16, kind="ExternalInput")
    out_d = nc.dram_tensor("out", [O, OH, OW], f32, kind="ExternalOutput")
    out_flat = out_d[:].flatten()

    with TileContext(nc) as tc:
        with (
            tc.tile_pool(name="const", bufs=1) as cpool,
            tc.tile_pool(name="vpool", bufs=4) as vpool,
            tc.tile_pool(name="opool", bufs=4) as opool,
            tc.tile_pool(name="zpool", bufs=3) as zpool,
            tc.tile_pool(name="affps", bufs=3, space="PSUM") as affps,
            tc.tile_pool(name="outps", bufs=2, space="PSUM") as outps,
        ):
            allin = cpool.tile([P, TOT], f16)
            osb_all = cpool.tile([P, NFREE], f32)
            # chunked load: weights first, then data/dep quarters so the
            # first pairs' compute overlaps the remaining transfers
            m17 = M0 + 17 * P
            nc.sync.dma_start(allin[:, m17 : m17 + P], allin_d[:, m17 : m17 + P])
            nc.sync.dma_start(allin[:, M0:m17], allin_d[:, M0:m17])
            nq4 = 8
            dq = (DWIN + nq4 - 1) // nq4
            zq = NFREE // nq4
            for cch in range(nq4):
                za, zb = Z0 + cch * zq, Z0 + (cch + 1) * zq
                nc.sync.dma_start(allin[:, za:zb], allin_d[:, za:zb])
                a, bnd = cch * dq, min(DWIN, (cch + 1) * dq)
                nc.sync.dma_start(allin[:, a:bnd], allin_d[:, a:bnd])

            def seg(off, size):
                return allin[:, off : off + size]

            def mk(base_ap, extra_off, dims):
                return AP(base_ap.tensor, base_ap.offset + extra_off, dims)

            # prologue: affinity for the whole image, pipelined per pair
            afft_all = cpool.tile([P, NFREE], f16)
            for u in range(NT // 2):
                base = u * 2 * NTILE
                zc2 = affps.tile([P, 2 * NTILE], f32, tag="affps")
                for h in range(2):
                    nc.tensor.matmul(
                        zc2[:, h * NTILE : (h + 1) * NTILE],
                        seg(M0 + 17 * P, P),
                        seg(Z0 + base + h * NTILE, NTILE),
                        start=True,
                        stop=True,
                    )
                nc.scalar.activation(
                    afft_all[:, base : base + 2 * NTILE], zc2[:],
                    AF.Abs, scale=-ALPHA,
                )
                nc.scalar.activation(
                    afft_all[:, base : base + 2 * NTILE],
                    afft_all[:, base : base + 2 * NTILE],
                    AF.Exp, scale=-1.0,
                )

            for u in range(NT // 2):
                base = u * 2 * NTILE
                afft = afft_all[:, base : base + 2 * NTILE]
                outp_a = outps.tile([P, NTILE], f32, tag="outp")
                outp_b = outps.tile([P, NTILE], f32, tag="outp")
                for idx, k in enumerate(range(9)):
                    i, j = TAPS[k]
                    shift = base + i * W + j
                    if k == 4:
                        rhs_a = seg(D0 + shift, NTILE)
                        rhs_b = seg(D0 + shift + NTILE, NTILE)
                    else:
                        jj = NC_KS.index(k)
                        ap2 = affps.tile([P, 2 * NTILE], f32, tag="affps")
                        for h in range(2):
                            nc.tensor.matmul(
                                ap2[:, h * NTILE : (h + 1) * NTILE],
                                seg(M0 + (9 + jj) * P, P),
                                afft[h * NTILE : (h + 1) * NTILE] if False else afft[:, h * NTILE : (h + 1) * NTILE],
                                start=True,
                                stop=True,
                            )
                        v2 = vpool.tile([P, 2 * NTILE], f16, tag="v")
                        if jj in (0, 3, 5):
                            ap_sb = zpool.tile([P, 2 * NTILE], f16, tag="apsb")
                            nc.scalar.copy(ap_sb[:], ap2[:])
                            nc.vector.tensor_tensor(
                                v2[:], seg(D0 + shift, 2 * NTILE), ap_sb[:],
                                AluOpType.mult,
                            )
                        else:
                            nc.vector.tensor_tensor(
                                v2[:], seg(D0 + shift, 2 * NTILE), ap2[:],
                                AluOpType.mult,
                            )
                        rhs_a = v2[:, 0:NTILE]
                        rhs_b = v2[:, NTILE : 2 * NTILE]
                    nc.tensor.matmul(
                        outp_a[:], seg(M0 + k * P, P), rhs_a,
                        start=(idx == 0), stop=(idx == 8),
                        skip_group_check=True,
                    )
                    nc.tensor.matmul(
                        outp_b[:], seg(M0 + k * P, P), rhs_b,
                        start=(idx == 0), stop=(idx == 8),
                        skip_group_check=True,
                    )
                nc.scalar.copy(osb_all[:, base : base + NTILE], outp_a[:])
                nc.scalar.copy(
                    osb_all[:, base + NTILE : base + 2 * NTILE], outp_b[:]
                )
                if True:
                    x0 = 4 * u
                    for q in range(NQ):
                        nrows = max(0, min(x0 + 4, OH - 32 * q) - x0)
                        if nrows == 0:
                            continue
                        src_ap = osb_all[16 * q : 16 * q + 16, :].rearrange(
                            "o (x y) -> o x y", y=W
                        )[:, x0 : x0 + nrows, 0:OW]
                        nc.sync.dma_start(
                            out_d[:, 32 * q + x0 : 32 * q + x0 + nrows, :], src_ap
                        )
    nc.compile()
    return nc


def _pack_inputs(data, depth, weight):
    """Build the [B, 128, TOT] packed input: data windows, shifted depth
    taps, and the weight/selection matrices."""
    HP = H + 3
    data_p = np.zeros((B, C, HP * W), np.float32)
    data_p[:, :, : H * W] = data.reshape(B, C, H * W)
    depth_p = np.zeros((B, HP * W), np.float32)
    depth_p[:, : H * W] = depth.reshape(B, H * W)

    wsmat = np.zeros((NBLK, P, P), np.float32)
    for k in range(9):
        i, j = TAPS[k]
        blk = weight[:, :, i, j].T  # [c, o]
        for q in range(NQ):
            wsmat[k, 16 * q : 16 * q + 16, 16 * q : 16 * q + 16] = blk
    for jj, k in enumerate(NC_KS):
        for q in range(NQ):
            wsmat[9 + jj, 16 * q + k, 16 * q : 16 * q + 16] = 1.0
    wsmat[17] = np.eye(P, dtype=np.float32)
    for q in range(NQ):
        wsmat[17, 16 * q + 4, 16 * q : 16 * q + 16] -= 1.0
    wsmat_flat = wsmat.transpose(1, 0, 2).reshape(P, NBLK * P)

    allin = np.zeros((B, P, TOT), np.float16)
    for q in range(NQ):
        for c in range(C):
            p = 16 * q + c
            s = 32 * q * W
            allin[:, p, D0 : D0 + DWIN] = data_p[:, c, s : s + DWIN]
        for k, (i, j) in enumerate(TAPS):
            p = 16 * q + k
            s = (32 * q + i) * W + j
            allin[:, p, Z0 : Z0 + NFREE] = depth_p[:, s : s + NFREE]
    allin[:, :, M0:] = wsmat_flat[None]
    return allin


def run(inputs, **spmd_kwargs):
    from concourse.bass_utils import run_bass_kernel_spmd

    data = np.asarray(inputs["data"], np.float32)
    depth = np.asarray(inputs["depth"], np.float32)
    weight = np.asarray(inputs["weight"], np.float32)
    allin = _pack_inputs(data, depth, weight)

    if "nc" not in _CACHE:
        _CACHE["nc"] = _build_nc()
    nc = _CACHE["nc"]

    in_maps = [{"allin": np.ascontiguousarray(allin[b])} for b in range(B)]
    res = run_bass_kernel_spmd(nc, in_maps, core_ids=list(range(B)), **spmd_kwargs)
    out = np.stack([res.results[b]["out"] for b in range(B)]).astype(np.float32)
    return out, res


def kernel(**inputs):
    out, _ = run(inputs)
    return out



# revision 5
# speedup vs baseline: 1.0919x; 1.0919x over previous
"""DepthConv kernel for Trainium2 (Bass/Tile), data-parallel over batch on 8 cores.

Problem: out[b,o,x,y] = sum_{c,k} w[o,c,k] * data[b,c,x+i,y+j] * aff[b,k,x,y]
         aff[b,k,x,y] = exp(-8.3*|depth[b,x+i,y+j] - depth[b,x+1,y+1]|), k=(i,j) in 3x3
Shapes: data [8,16,256,256], depth [8,1,256,256], weight [16,16,3,3] -> out [8,16,254,254]

Per-core layout (1 image/core): partitions = (strip q=0..7, channel c=0..15).
Each strip covers 32 output rows; free dim n = xl*256+y (flat, row-wrapping).
 - 3x3 taps become pure free-dim shifts (i*256+j) of one resident data tile.
 - Per-tap matmul uses block-diagonal weights [(q,c),(q,o)] so all 8 strips'
   channel contractions run in one full-width 128x128 matmul; 9 taps
   PSUM-accumulate.
 - Main loop runs at 512-px (2 output rows/strip) granularity and computes the
   affinity inline (center-select matmul + ACT abs/exp), immediately followed
   by 8 tap-broadcast matmuls, DVE/ACT multiplies, and 9 accumulating output
   matmuls; the output tile goes PSUM -> DRAM in a single strided DMA, so no
   SBUF output staging and no serial affinity prologue.
 - Tap multiplies are balanced across engines: 3 taps multiply straight from
   PSUM on the DVE, 5 taps get an ACT f16 copy first so the DVE runs in 2x
   mode; ACT also does the abs/exp.
"""

import numpy as np

B, C, H, W = 8, 16, 256, 256
O, KH, KW = 16, 3, 3
ALPHA = 8.3
OH, OW = H - KH + 1, W - KW + 1  # 254, 254
P = 128
NQ, QROWS = 8, 32           # strips, output rows per strip
NFREE = QROWS * W           # 8192 flat pixels per strip (incl. y>=254 garbage)
NTILE = 512                 # 2 output rows per tile
NT = NFREE // NTILE         # 16 n-tiles
DWIN = 34 * W + 16          # data window: 34 rows halo + shift pad
TAPS = [(i, j) for i in range(KH) for j in range(KW)]
NC_KS = [k for k in range(9) if k != 4]  # non-center taps
NBLK = 18                   # 9 weight blocks + 8 tap-select + 1 center-select
D0 = 0                      # data window offset in the packed tensor
Z0 = DWIN                   # dep_t offset
M0 = DWIN + NFREE           # wsmat offset
TOT = DWIN + NFREE + NBLK * P
DIRECT_KS = (0, 5, 7)       # taps multiplied straight from PSUM on DVE

_CACHE = {}


def _build_nc():
    import concourse.bass as bass
    import concourse.bacc as bacc
    import concourse.mybir as mybir
    from concourse.tile import TileContext
    from concourse.alu_op_type import AluOpType
    from concourse.bass_types import AP

    f32 = mybir.dt.float32
    f16 = mybir.dt.float16
    AF = mybir.ActivationFunctionType

    nc = bacc.Bacc(None, target_bir_lowering=False)
    allin_d = nc.dram_tensor("allin", [P, TOT], f16, kind="ExternalInput")
    out_d = nc.dram_tensor("out", [O, OH, OW], f16, kind="ExternalOutput")

    with TileContext(nc) as tc:
        with (
            tc.tile_pool(name="const", bufs=1) as cpool,
            tc.tile_pool(name="apool", bufs=3) as apool,
            tc.tile_pool(name="vpool", bufs=6) as vpool,
            tc.tile_pool(name="spool", bufs=4) as spool,
            tc.tile_pool(name="zcps", bufs=1, space="PSUM") as zcps,
            tc.tile_pool(name="affps", bufs=3, space="PSUM") as affps,
            tc.tile_pool(name="outps", bufs=3, space="PSUM") as outps,
        ):
            allin = cpool.tile([P, TOT], f16)
            # chunked load: selection+weight matrices first, then interleaved
            # depth/data quarters so the first tiles' compute overlaps the
            # remaining transfers
            m17 = M0 + 17 * P
            nc.sync.dma_start(allin[:, m17 : m17 + P], allin_d[:, m17 : m17 + P])
            nc.sync.dma_start(allin[:, M0:m17], allin_d[:, M0:m17])
            nq4 = 8
            dq = (DWIN + nq4 - 1) // nq4
            zq = NFREE // nq4
            for cch in range(nq4):
                za, zb = Z0 + cch * zq, Z0 + (cch + 1) * zq
                nc.sync.dma_start(allin[:, za:zb], allin_d[:, za:zb])
                a, bnd = cch * dq, min(DWIN, (cch + 1) * dq)
                nc.sync.dma_start(allin[:, a:bnd], allin_d[:, a:bnd])

            def seg(off, size):
                return allin[:, off : off + size]

            for u in range(NT):
                base = u * NTILE
                # inline affinity: center-diff matmul + abs/exp
                zc = zcps.tile([P, NTILE], f32, tag="zc")
                nc.tensor.matmul(
                    zc[:], seg(M0 + 17 * P, P), seg(Z0 + base, NTILE),
                    start=True, stop=True,
                )
                afft = apool.tile([P, NTILE], f16, tag="afft")
                nc.scalar.activation(afft[:], zc[:], AF.Abs, scale=-ALPHA)
                nc.scalar.activation(afft[:], afft[:], AF.Exp, scale=-1.0)

                outp = outps.tile([P, NTILE], f32, tag="outp")
                taps = NC_KS  # 8 non-center taps
                ap2s = {}

                def bcast(k):
                    jj = NC_KS.index(k)
                    ap2 = affps.tile([P, NTILE], f32, tag="ap2")
                    nc.tensor.matmul(
                        ap2[:], seg(M0 + (9 + jj) * P, P), afft[:],
                        start=True, stop=True, skip_group_check=True,
                    )
                    ap2s[k] = ap2

                # PE lookahead: 2 broadcasts in flight before each output
                bcast(taps[0])
                bcast(taps[1])
                # center tap opens the accumulation (needs no multiply)
                nc.tensor.matmul(
                    outp[:], seg(M0 + 4 * P, P),
                    seg(D0 + base + 1 * W + 1, NTILE),
                    start=True, stop=False, skip_group_check=True,
                )
                for idx, k in enumerate(taps):
                    if idx + 2 < len(taps):
                        bcast(taps[idx + 2])
                    i, j = TAPS[k]
                    shift = base + i * W + j
                    ap2 = ap2s.pop(k)
                    v2 = vpool.tile([P, NTILE], f16, tag="v")
                    if k in DIRECT_KS:
                        nc.vector.tensor_tensor(
                            v2[:], seg(D0 + shift, NTILE), ap2[:],
                            AluOpType.mult,
                        )
                    else:
                        ap_sb = spool.tile([P, NTILE], f16, tag="apsb")
                        nc.scalar.copy(ap_sb[:], ap2[:])
                        nc.vector.tensor_tensor(
                            v2[:], seg(D0 + shift, NTILE), ap_sb[:],
                            AluOpType.mult,
                        )
                    nc.tensor.matmul(
                        outp[:], seg(M0 + k * P, P), v2[:],
                        start=False, stop=(idx == len(taps) - 1),
                        skip_group_check=True,
                    )

                # ACT copy PSUM -> packed f16 SBUF (2 rows x 254 contiguous),
                # then one strided DMA (partitions (q,o) -> dst dims)
                osb = spool.tile([P, 2 * OW], f16, tag="osb")
                nc.scalar.copy(
                    osb[:],
                    outp[:].rearrange("p (x y) -> p x y", y=W)[:, :, 0:OW],
                )
                x0 = 2 * u
                nq_full = NQ if x0 + 2 <= 30 else NQ - 1
                dst_ap = AP(
                    out_d[:].tensor,
                    x0 * OW,
                    [[QROWS * OW, nq_full], [OH * OW, O], [1, 2 * OW]],
                )
                nc.sync.dma_start(dst_ap, osb[0 : 16 * nq_full, :])
                if nq_full < NQ:
                    # strip 7 has only 30 valid output rows (224..253)
                    nrows = max(0, min(x0 + 2, OH - QROWS * 7) - x0)
                    if nrows:
                        nc.sync.dma_start(
                            out_d[:, QROWS * 7 + x0 : QROWS * 7 + x0 + nrows, :],
                            osb[16 * 7 : 16 * 7 + 16, 0 : nrows * OW],
                        )
    nc.compile()
    return nc


def _pack_inputs(data, depth, weight):
    """Build the [B, 128, TOT] packed input: data windows, shifted depth
    taps, and the weight/selection matrices."""
    HP = H + 3
    data_p = np.zeros((B, C, HP * W), np.float32)
    data_p[:, :, : H * W] = data.reshape(B, C, H * W)
    depth_p = np.zeros((B, HP * W), np.float32)
    depth_p[:, : H * W] = depth.reshape(B, H * W)

    wsmat = np.zeros((NBLK, P, P), np.float32)
    for k in range(9):
        i, j = TAPS[k]
        blk = weight[:, :, i, j].T  # [c, o]
        for q in range(NQ):
            wsmat[k, 16 * q : 16 * q + 16, 16 * q : 16 * q + 16] = blk
    for jj, k in enumerate(NC_KS):
        for q in range(NQ):
            wsmat[9 + jj, 16 * q + k, 16 * q : 16 * q + 16] = 1.0
    wsmat[17] = np.eye(P, dtype=np.float32)
    for q in range(NQ):
        wsmat[17, 16 * q + 4, 16 * q : 16 * q + 16] -= 1.0
    wsmat_flat = wsmat.transpose(1, 0, 2).reshape(P, NBLK * P)

    allin = np.zeros((B, P, TOT), np.float16)
    for q in range(NQ):
        for c in range(C):
            p = 16 * q + c
            s = 32 * q * W
            allin[:, p, D0 : D0 + DWIN] = data_p[:, c, s : s + DWIN]
        for k, (i, j) in enumerate(TAPS):
            p = 16 * q + k
            s = (32 * q + i) * W + j
            allin[:, p, Z0 : Z0 + NFREE] = depth_p[:, s : s + NFREE]
    allin[:, :, M0:] = wsmat_flat[None]
    return allin


def run(inputs, **spmd_kwargs):
    from concourse.bass_utils import run_bass_kernel_spmd

    data = np.asarray(inputs["data"], np.float32)
    depth = np.asarray(inputs["depth"], np.float32)
    weight = np.asarray(inputs["weight"], np.float32)
    allin = _pack_inputs(data, depth, weight)

    if "nc" not in _CACHE:
        _CACHE["nc"] = _build_nc()
    nc = _CACHE["nc"]

    in_maps = [{"allin": np.ascontiguousarray(allin[b])} for b in range(B)]
    res = run_bass_kernel_spmd(nc, in_maps, core_ids=list(range(B)), **spmd_kwargs)
    out = np.stack([res.results[b]["out"] for b in range(B)]).astype(np.float32)
    return np.ascontiguousarray(out), res


def kernel(**inputs):
    out, _ = run(inputs)
    return out
